# revision 1
# baseline (speedup 1.0000x reference)
"""CosineTransformerBlock Trainium2 kernel (8 NeuronCores, SPMD, no collectives).

Sharding: core c handles batch b = c // 2 and query-token rows
[ (c % 2) * 1024 : (c % 2) * 1024 + 1024 ] of that batch.  K/V work for a
batch is duplicated across the 2 cores that share it (cheaper than pair
collectives on this chip).

Key algebraic transform: cosine attention has no softmax, so
    (qn @ kn^T) @ v  ==  qn @ (kn^T @ v)
which turns the O(N^2) attention into two tiny per-head [64,64] matmuls.

Layout strategy (per core):
  - activations are token-major [tok, feat] so LayerNorm / l2-norm use
    bn_stats + per-partition scalar ops;
  - matmul lhsT operands are produced by casting to bf16 and XBAR
    DMA-transposing 128x128 blocks (free: runs on DMA engines);
  - all matmuls are bf16 with fp32 PSUM accumulation;
  - LN affine (g, b) is folded into the following weight matrix on the host:
      LN(x) @ W = std(x) @ (g[:, None] * W) + (b @ W)
    the (b @ W) row term is added via a K=1 ones-matmul into the same PSUM
    accumulation group (emitted only when the row is nonzero).
"""

import os
import sys

sys.path.insert(0, "/opt/trn_rl_repo")

import numpy as np
import ml_dtypes

# ---- problem shapes (hardcoded per contract) ----
B, N, D = 4, 2048, 1024
H, DH = 16, 64
INNER = H * DH  # 1024
MLP = 4096
EPS = 1e-5
NCORES = 8
TQ = N // 2  # 1024 query tokens per core
TKV = N  # 2048 kv tokens per core
P = 128
DC = D // P  # 8 chunks of the model dim
IC = INNER // P  # 8
MC = MLP // P  # 32
NQT = TQ // P  # 8 q token tiles
NKT = TKV // P  # 16 kv token tiles

BF16 = None  # set lazily (mybir import)
F32 = None


def _dt():
    global BF16, F32
    import concourse.mybir as mybir

    BF16 = mybir.dt.bfloat16
    F32 = mybir.dt.float32
    return mybir


def _ln_stats_ops(nc, pool, x_tile, ntok, dfree, eps_tile):
    """bn_stats/bn_aggr over free dim -> (rs, neg_mu_rs) [ntok,1] fp32."""
    import concourse.mybir as mybir

    nsub = (dfree + 511) // 512
    sub = dfree // nsub
    stats = pool.tile([P, nsub, 6], F32, tag="ln_stats")
    xv = x_tile.rearrange("p (s f) -> p s f", s=nsub)
    for s in range(nsub):
        nc.vector.bn_stats(out=stats[:ntok, s, :], in_=xv[:ntok, s, :])
    mv = pool.tile([P, 2], F32, tag="ln_mv")
    nc.vector.bn_aggr(out=mv[:ntok], in_=stats[:ntok])
    rs = pool.tile([P, 1], F32, tag="ln_rs")
    # rs = 1/sqrt(var + eps)
    nc.scalar.activation(
        out=rs[:ntok],
        in_=mv[:ntok, 1:2],
        func=mybir.ActivationFunctionType.Sqrt,
        bias=eps_tile[:ntok],
        scale=1.0,
    )
    nc.vector.reciprocal(out=rs[:ntok], in_=rs[:ntok])
    nmu = pool.tile([P, 1], F32, tag="ln_nmu")
    # nmu = -mu * rs
    nc.vector.tensor_scalar(
        out=nmu[:ntok],
        in0=mv[:ntok, 0:1],
        scalar1=rs[:ntok],
        scalar2=-1.0,
        op0=mybir.AluOpType.mult,
        op1=mybir.AluOpType.mult,
    )
    return rs, nmu


def build_nc(bias_rows):
    """Build the SPMD program. bias_rows: dict of host-computed fp32 rows
    (bq, bk, bv, bo, b2: [dim] arrays) - a K=1 ones-matmul is emitted for
    each nonzero row."""
    mybir = _dt()
    import concourse.bass as bass
    import concourse.tile as tile
    from concourse import bacc

    AF = mybir.ActivationFunctionType
    ALU = mybir.AluOpType

    nc = bacc.Bacc("TRN2", target_bir_lowering=False, debug=False, num_devices=NCORES)

    # ---- DRAM I/O ----
    Qd = nc.dram_tensor("q_tok", [TQ, D], F32, kind="ExternalInput").ap()
    Kd = nc.dram_tensor("k_tok", [TKV, D], BF16, kind="ExternalInput").ap()
    Vd = nc.dram_tensor("v_tok", [TKV, D], BF16, kind="ExternalInput").ap()
    wq_d = nc.dram_tensor("wq", [D, INNER], BF16, kind="ExternalInput").ap()
    wk_d = nc.dram_tensor("wk", [D, INNER], BF16, kind="ExternalInput").ap()
    wv_d = nc.dram_tensor("wv", [D, INNER], BF16, kind="ExternalInput").ap()
    wo_d = nc.dram_tensor("wo", [INNER, D], BF16, kind="ExternalInput").ap()
    w1_d = nc.dram_tensor("w1", [MC * P, DC * P], BF16, kind="ExternalInput").ap()
    w2_d = nc.dram_tensor("w2", [MLP, D], BF16, kind="ExternalInput").ap()
    bff1_d = nc.dram_tensor("bff1", [P, MC], F32, kind="ExternalInput").ap()
    brow_d = {}
    for name in ("bq", "bk", "bv", "bo", "b2"):
        if np.any(bias_rows[name]):
            brow_d[name] = nc.dram_tensor(
                "brow_" + name, [1, bias_rows[name].shape[0]], BF16,
                kind="ExternalInput",
            ).ap()
    Yd = nc.dram_tensor("y", [TQ, D], F32, kind="ExternalOutput").ap()
    DBG = bool(os.environ.get("BASS_DEBUG_KERNEL"))
    dbg = {}
    if DBG:
        for nm, shp in [("dbg_kn", [P, INNER]), ("dbg_v", [P, INNER]),
                        ("dbg_M", [P, IC * DH]), ("dbg_x", [P, D]),
                        ("dbg_qn", [P, INNER]), ("dbg_aT", [P, IC * P])]:
            dbg[nm] = nc.dram_tensor(nm, shp, F32, kind="ExternalOutput").ap()

    Qt = Qd.rearrange("(t p) d -> t p d", p=P)
    Kt = Kd.rearrange("(t p) d -> t p d", p=P)
    Vt = Vd.rearrange("(t p) d -> t p d", p=P)
    Yt = Yd.rearrange("(t p) d -> t p d", p=P)
    # weight DRAM views: [P, chunk, cols]
    wq_v = wq_d.rearrange("(c p) n -> p c n", p=P)
    wk_v = wk_d.rearrange("(c p) n -> p c n", p=P)
    wv_v = wv_d.rearrange("(c p) n -> p c n", p=P)
    wo_v = wo_d.rearrange("(c p) n -> p c n", p=P)
    w1_v = w1_d.rearrange("(m p) (c q) -> m p c q", p=P, c=DC)
    w2_v = w2_d.rearrange("(c p) n -> p c n", p=P)

    with tile.TileContext(nc) as tc:
        with tc.tile_pool(name="singles", bufs=1) as singles:
            # resident weights
            wq_sb = singles.tile([P, DC, INNER], BF16)
            wk_sb = singles.tile([P, DC, INNER], BF16)
            wv_sb = singles.tile([P, DC, INNER], BF16)
            wo_sb = singles.tile([P, IC, D], BF16)
            for c in range(DC):
                nc.gpsimd.dma_start(wk_sb[:, c, :], wk_v[:, c, :])
            nc.sync.dma_start(wv_sb[:], wv_v[:])
            nc.gpsimd.dma_start(wq_sb[:], wq_v[:])
            nc.gpsimd.dma_start(wo_sb[:], wo_v[:])
            bff1_sb = singles.tile([P, MC], F32)
            nc.sync.dma_start(bff1_sb[:], bff1_d[:])
            eps_tile = singles.tile([P, 1], F32)
            nc.vector.memset(eps_tile[:], EPS)
            ones_row = singles.tile([1, P], BF16)
            nc.vector.memset(ones_row[:], 1.0)
            brow_sb = {}
            for name, ap in brow_d.items():
                t = singles.tile([1, ap.shape[1]], BF16, tag="brow_" + name)
                nc.sync.dma_start(t[:], ap[:])
                brow_sb[name] = t
            # residual / LN2 source
            x_sb = singles.tile([P, NQT, D], F32)
            # head-pair attention matrices: M_sb[:, pr, :] is
            # blockdiag(M_2pr, M_2pr+1); off-diagonal junk stays zero
            M_sb = singles.tile([P, IC, P], BF16)
            nc.vector.memset(M_sb[:], 0.0)

            def bias_mm(ps, name, lo, hi, start):
                """Accumulate bias row[lo:hi] into psum ps via K=1 matmul."""
                if name in brow_sb:
                    nc.tensor.matmul(
                        ps,
                        ones_row[:, : ps.shape[0]],
                        brow_sb[name][:, lo:hi],
                        start=start,
                        stop=False,
                        skip_group_check=True,
                    )
                    return False  # start consumed
                return start

            # ---------------- Phase 1: K/V -> M_h ----------------
            with (
                tc.tile_pool(name="kv_io", bufs=4) as kv_io,
                tc.tile_pool(name="kv_mid", bufs=3) as kv_mid,
                tc.tile_pool(name="kv_stats", bufs=4) as kv_stats,
                tc.tile_pool(name="kv_ps", bufs=6, space="PSUM") as kv_ps,
                tc.tile_pool(name="m_ps", bufs=1, space="PSUM") as m_ps_pool,
            ):
                M_ps = m_ps_pool.tile([P, IC, P], F32)
                for t in range(NKT):
                    kn_bf = None
                    v_bf = None
                    for which in ("k", "v"):
                        src = Kt[t] if which == "k" else Vt[t]
                        w_sb = wk_sb if which == "k" else wv_sb
                        bname = "bk" if which == "k" else "bv"
                        x_in = kv_io.tile([P, D], BF16, tag="kv_in")
                        nc.sync.dma_start(x_in[:], src[:])
                        rs, nmu = _ln_stats_ops(nc, kv_stats, x_in, P, D, eps_tile)
                        xn = kv_mid.tile([P, D], BF16, tag="kv_std")
                        nc.scalar.activation(
                            out=xn[:], in_=x_in[:], func=AF.Identity, bias=nmu[:], scale=rs[:]
                        )
                        xnT = kv_mid.tile([P, DC, P], BF16, tag="kv_xnT")
                        for c in range(DC):
                            nc.sync.dma_start(
                                xnT[:, c, :], xn[:, c * P : (c + 1) * P], transpose=True
                            )
                        # projection: [tok, INNER] in 2 groups of 512
                        pss = []
                        for g in range(2):
                            ps = kv_ps.tile([P, 512], F32, tag="kv_proj")
                            pss.append(ps)
                        for c in range(DC):
                            for g in range(2):
                                nc.tensor.matmul(
                                    pss[g][:],
                                    xnT[:, c, :],
                                    w_sb[:, c, g * 512 : (g + 1) * 512],
                                    start=(c == 0),
                                    stop=(c == DC - 1) and (bname not in brow_sb),
                                )
                        for g in range(2):
                            if bname in brow_sb:
                                nc.tensor.matmul(
                                    pss[g][:],
                                    ones_row[:],
                                    brow_sb[bname][:, g * 512 : (g + 1) * 512],
                                    start=False,
                                    stop=True,
                                    skip_group_check=True,
                                )
                        if which == "v":
                            v_bf = kv_mid.tile([P, INNER], BF16, tag="v_bf")
                            for g in range(2):
                                nc.scalar.activation(
                                    out=v_bf[:, g * 512 : (g + 1) * 512],
                                    in_=pss[g][:],
                                    func=AF.Copy,
                                )
                        else:
                            # l2-normalize per head
                            kn_bf = kv_mid.tile([P, H, DH], BF16, tag="kn_bf")
                            for g in range(2):
                                sq = kv_mid.tile([P, 512], F32, tag="kv_sq")
                                nc.scalar.activation(
                                    out=sq[:], in_=pss[g][:], func=AF.Square
                                )
                                ss = kv_stats.tile([P, 8, 1], F32, tag="l2_ss")
                                nc.vector.reduce_sum(
                                    out=ss[:],
                                    in_=sq.rearrange("p (h f) -> p h f", h=8),
                                    axis=mybir.AxisListType.X,
                                )
                                rn = kv_stats.tile([P, 8, 1], F32, tag="l2_rn")
                                nc.scalar.activation(
                                    out=rn[:], in_=ss[:], func=AF.Sqrt
                                )
                                nc.vector.tensor_scalar_max(
                                    out=rn[:], in0=rn[:], scalar1=1e-12
                                )
                                nc.vector.reciprocal(out=rn[:], in_=rn[:])
                                nc.vector.tensor_tensor(
                                    out=kn_bf[:, g * 8 : (g + 1) * 8, :],
                                    in0=pss[g].rearrange("p (h f) -> p h f", h=8),
                                    in1=rn.to_broadcast([P, 8, DH]),
                                    op=ALU.mult,
                                )
                    if DBG and t == 0:
                        knf = kv_mid.tile([P, INNER], F32, tag="dbg_knf")
                        nc.vector.tensor_copy(out=knf.rearrange("p (h f) -> p h f", h=H), in_=kn_bf[:])
                        nc.sync.dma_start(dbg["dbg_kn"][:], knf[:])
                        vf = kv_mid.tile([P, INNER], F32, tag="dbg_vf")
                        nc.vector.tensor_copy(out=vf[:], in_=v_bf[:])
                        nc.sync.dma_start(dbg["dbg_v"][:], vf[:])
                    # M_h accumulation: M[h] += kn_h^T @ v_h
                    # Heads are processed in pairs: one [128,128] matmul per
                    # pair computes blockdiag(M_2pr, M_2pr+1) plus junk
                    # off-diagonal blocks (discarded at evac). start=True zeroes
                    # the whole 2KB PSUM zero-region (= 4 pair blocks), so only
                    # the first pair per region starts the group and only the
                    # last stops it.
                    kn_flat = kn_bf.rearrange("p h f -> p (h f)")
                    for pr in range(IC):
                        nc.tensor.matmul(
                            M_ps[:, pr, :],
                            kn_flat[:, pr * P : (pr + 1) * P],
                            v_bf[:, pr * P : (pr + 1) * P],
                            start=(t == 0 and pr % 4 == 0),
                            stop=(t == NKT - 1 and pr % 4 == 3),
                            skip_group_check=True,
                        )
                for po in (0, 64):
                    nc.scalar.activation(
                        out=M_sb[po : po + 64, :, po : po + 64],
                        in_=M_ps[po : po + 64, :, po : po + 64],
                        func=AF.Copy,
                    )
            if DBG:
                with tc.tile_pool(name="dbgp", bufs=1) as dbgp:
                    mf = dbgp.tile([P, IC, DH], F32)
                    for po in (0, 64):
                        nc.vector.tensor_copy(
                            out=mf[po : po + 64, :, :],
                            in_=M_ps[po : po + 64, :, po : po + 64],
                        )
                    nc.sync.dma_start(dbg["dbg_M"].rearrange("p (c f) -> p c f", c=IC)[:], mf[:])

            # ---------------- Phase 2: Q -> attn -> x ----------------
            with (
                tc.tile_pool(name="q_io", bufs=3) as q_io,
                tc.tile_pool(name="q_mid", bufs=3) as q_mid,
                tc.tile_pool(name="q_stats", bufs=4) as q_stats,
                tc.tile_pool(name="q_ps", bufs=2, space="PSUM") as q_ps,
                tc.tile_pool(name="x_ps", bufs=2, space="PSUM") as x_ps,
                tc.tile_pool(name="at_ps", bufs=2, space="PSUM") as at_ps,
            ):
                for t in range(NQT):
                    q_in = q_io.tile([P, D], F32, tag="q_in")
                    nc.sync.dma_start(q_in[:], Qt[t][:])
                    rs, nmu = _ln_stats_ops(nc, q_stats, q_in, P, D, eps_tile)
                    qn_std = q_mid.tile([P, D], BF16, tag="q_std")
                    nc.scalar.activation(
                        out=qn_std[:], in_=q_in[:], func=AF.Identity, bias=nmu[:], scale=rs[:]
                    )
                    qnT = q_mid.tile([P, DC, P], BF16, tag="q_xnT")
                    for c in range(DC):
                        nc.sync.dma_start(
                            qnT[:, c, :], qn_std[:, c * P : (c + 1) * P], transpose=True
                        )
                    pss = []
                    for g in range(2):
                        ps = q_ps.tile([P, 512], F32, tag="q_proj")
                        pss.append(ps)
                    for c in range(DC):
                        for g in range(2):
                            nc.tensor.matmul(
                                pss[g][:],
                                qnT[:, c, :],
                                wq_sb[:, c, g * 512 : (g + 1) * 512],
                                start=(c == 0),
                                stop=(c == DC - 1) and ("bq" not in brow_sb),
                            )
                    for g in range(2):
                        if "bq" in brow_sb:
                            nc.tensor.matmul(
                                pss[g][:],
                                ones_row[:],
                                brow_sb["bq"][:, g * 512 : (g + 1) * 512],
                                start=False,
                                stop=True,
                                skip_group_check=True,
                            )
                    # l2-normalize q per head -> qn bf16
                    qn_bf = q_mid.tile([P, H, DH], BF16, tag="qn_bf")
                    for g in range(2):
                        sq = q_mid.tile([P, 512], F32, tag="q_sq")
                        nc.scalar.activation(out=sq[:], in_=pss[g][:], func=AF.Square)
                        ss = q_stats.tile([P, 8, 1], F32, tag="ql2_ss")
                        nc.vector.reduce_sum(
                            out=ss[:],
                            in_=sq.rearrange("p (h f) -> p h f", h=8),
                            axis=mybir.AxisListType.X,
                        )
                        rn = q_stats.tile([P, 8, 1], F32, tag="ql2_rn")
                        nc.scalar.activation(out=rn[:], in_=ss[:], func=AF.Sqrt)
                        nc.vector.tensor_scalar_max(out=rn[:], in0=rn[:], scalar1=1e-12)
                        nc.vector.reciprocal(out=rn[:], in_=rn[:])
                        nc.vector.tensor_tensor(
                            out=qn_bf[:, g * 8 : (g + 1) * 8, :],
                            in0=pss[g].rearrange("p (h f) -> p h f", h=8),
                            in1=rn.to_broadcast([P, 8, DH]),
                            op=ALU.mult,
                        )
                    # transpose qn -> [INNER, tok] feature-major
                    qn_flat = qn_bf.rearrange("p h f -> p (h f)")
                    qnT2 = q_mid.tile([P, IC, P], BF16, tag="qnT2")
                    for c in range(IC):
                        nc.sync.dma_start(
                            qnT2[:, c, :], qn_flat[:, c * P : (c + 1) * P], transpose=True
                        )
                    # attn^T[h] = M_h^T @ qn_h^T  -> [INNER, tok] chunks
                    a_ps = at_ps.tile([P, IC, P], F32, tag="attn_ps")
                    for pr in range(IC):
                        nc.tensor.matmul(
                            a_ps[:, pr, :],
                            M_sb[:, pr, :],
                            qnT2[:, pr, :],
                            start=True,
                            stop=True,
                            skip_group_check=True,
                        )
                    aT_bf = q_mid.tile([P, IC, P], BF16, tag="aT_bf")
                    nc.scalar.activation(out=aT_bf[:], in_=a_ps[:], func=AF.Copy)
                    if DBG and t == 0:
                        qnf = q_mid.tile([P, INNER], F32, tag="dbg_qnf")
                        nc.vector.tensor_copy(out=qnf.rearrange("p (h f) -> p h f", h=H), in_=qn_bf[:])
                        nc.sync.dma_start(dbg["dbg_qn"][:], qnf[:])
                        atf = q_mid.tile([P, IC, P], F32, tag="dbg_atf")
                        nc.vector.tensor_copy(out=atf[:], in_=a_ps[:])
                        nc.sync.dma_start(dbg["dbg_aT"].rearrange("p (c f) -> p c f", c=IC)[:], atf[:])
                    # x = Q + attn @ wo (+bo)
                    xps = []
                    for g in range(2):
                        ps = x_ps.tile([P, 512], F32, tag="x_proj")
                        xps.append(ps)
                    for c in range(IC):
                        for g in range(2):
                            nc.tensor.matmul(
                                xps[g][:],
                                aT_bf[:, c, :],
                                wo_sb[:, c, g * 512 : (g + 1) * 512],
                                start=(c == 0),
                                stop=(c == IC - 1) and ("bo" not in brow_sb),
                            )
                    for g in range(2):
                        if "bo" in brow_sb:
                            nc.tensor.matmul(
                                xps[g][:],
                                ones_row[:],
                                brow_sb["bo"][:, g * 512 : (g + 1) * 512],
                                start=False,
                                stop=True,
                                skip_group_check=True,
                            )
                        nc.vector.tensor_tensor(
                            out=x_sb[:, t, g * 512 : (g + 1) * 512],
                            in0=xps[g][:],
                            in1=q_in[:, g * 512 : (g + 1) * 512],
                            op=ALU.add,
                        )

            if DBG:
                nc.sync.dma_start(dbg["dbg_x"][:], x_sb[:, 0, :])
            # ---------------- Phase 3: FFN (groups of 4 token tiles) ----------------
            with (
                tc.tile_pool(name="f_mid", bufs=3) as f_mid,
                tc.tile_pool(name="f_w", bufs=3) as f_w,
                tc.tile_pool(name="f_h", bufs=1) as f_h,
                tc.tile_pool(name="f_stats", bufs=4) as f_stats,
                tc.tile_pool(name="f_out", bufs=3) as f_out,
                tc.tile_pool(name="h_ps", bufs=2, space="PSUM") as h_ps,
                tc.tile_pool(name="y_ps", bufs=4, space="PSUM") as y_ps,
            ):
                GT = 2  # token tiles per FFN group
                for grp in range(NQT // GT):
                    xnT4 = f_mid.tile([P, DC, GT * P], BF16, tag="xnT4")
                    for tt in range(GT):
                        t = grp * GT + tt
                        rs, nmu = _ln_stats_ops(
                            nc, f_stats, x_sb[:, t, :], P, D, eps_tile
                        )
                        xn = f_mid.tile([P, D], BF16, tag="f_std")
                        nc.scalar.activation(
                            out=xn[:],
                            in_=x_sb[:, t, :],
                            func=AF.Identity,
                            bias=nmu[:],
                            scale=rs[:],
                        )
                        for c in range(DC):
                            nc.sync.dma_start(
                                xnT4[:, c, tt * P : (tt + 1) * P],
                                xn[:, c * P : (c + 1) * P],
                                transpose=True,
                            )
                    # h^T = gelu(w1^T @ xn^T + b1) : feature-major [MLP, 4*128]
                    h4 = f_h.tile([P, MC, GT * P], BF16, tag="h4")
                    for m in range(MC):
                        w1t = f_w.tile([P, DC, P], BF16, tag="w1t")
                        nc.scalar.dma_start(w1t[:], w1_v[m])
                        hp = h_ps.tile([P, GT * P], F32, tag="h_ps_t")
                        for c in range(DC):
                            nc.tensor.matmul(
                                hp[:],
                                w1t[:, c, :],
                                xnT4[:, c, :],
                                start=(c == 0),
                                stop=(c == DC - 1),
                            )
                        nc.scalar.activation(
                            out=h4[:, m, :],
                            in_=hp[:],
                            func=AF.Gelu,
                            bias=bff1_sb[:, m : m + 1],
                            scale=1.0,
                        )
                    # y = x + h @ w2 (+ b2)
                    yps = [
                        [
                            y_ps.tile(
                                [P, 512], F32, tag="y_ps_t", name=f"yps_{tt}_{g}"
                            )
                            for g in range(2)
                        ]
                        for tt in range(GT)
                    ]
                    for m in range(MC):
                        w2t = f_w.tile([P, D], BF16, tag="w2t")
                        nc.gpsimd.dma_start(w2t[:], w2_v[:, m, :])
                        for tt in range(GT):
                            for g in range(2):
                                nc.tensor.matmul(
                                    yps[tt][g][:],
                                    h4[:, m, tt * P : (tt + 1) * P],
                                    w2t[:, g * 512 : (g + 1) * 512],
                                    start=(m == 0),
                                    stop=(m == MC - 1) and ("b2" not in brow_sb),
                                )
                    for tt in range(GT):
                        t = grp * GT + tt
                        for g in range(2):
                            if "b2" in brow_sb:
                                nc.tensor.matmul(
                                    yps[tt][g][:],
                                    ones_row[:],
                                    brow_sb["b2"][:, g * 512 : (g + 1) * 512],
                                    start=False,
                                    stop=True,
                                    skip_group_check=True,
                                )
                            y_out = f_out.tile([P, 512], F32, tag="y_out")
                            nc.vector.tensor_tensor(
                                out=y_out[:],
                                in0=yps[tt][g][:],
                                in1=x_sb[:, t, g * 512 : (g + 1) * 512],
                                op=ALU.add,
                            )
                            nc.sync.dma_start(
                                Yt[t][:, g * 512 : (g + 1) * 512], y_out[:]
                            )

    nc.compile()
    return nc


def prep_inputs(inputs):
    """Host-side shard + weight folding. Returns (in_maps, bias_rows)."""
    f32 = np.float32
    bf = ml_dtypes.bfloat16
    g1 = np.asarray(inputs["ln1_g"], f32)
    b1ln = np.asarray(inputs["ln1_b"], f32)
    g2 = np.asarray(inputs["ln2_g"], f32)
    b2ln = np.asarray(inputs["ln2_b"], f32)
    wq = np.asarray(inputs["wq"], f32)
    wk = np.asarray(inputs["wk"], f32)
    wv = np.asarray(inputs["wv"], f32)
    wo = np.asarray(inputs["wo"], f32)
    w1 = np.asarray(inputs["w1"], f32)
    w2 = np.asarray(inputs["w2"], f32)

    bias_rows = {
        "bq": (b1ln @ wq).astype(f32),
        "bk": (b1ln @ wk).astype(f32),
        "bv": (b1ln @ wv).astype(f32),
        "bo": np.asarray(inputs["bo"], f32),
        "b2": np.asarray(inputs["b2"], f32),
    }
    bff1 = (b2ln @ w1 + np.asarray(inputs["b1"], f32)).astype(f32)
    bff1_tile = np.ascontiguousarray(bff1.reshape(MC, P).T)  # [P, MC]

    wq_b = np.ascontiguousarray((g1[:, None] * wq).astype(bf))
    wk_b = np.ascontiguousarray((g1[:, None] * wk).astype(bf))
    wv_b = np.ascontiguousarray((g1[:, None] * wv).astype(bf))
    wo_b = np.ascontiguousarray(wo.astype(bf))
    w1g = (g2[:, None] * w1).astype(bf)
    # pack w1 so each streamed [P, DC*P] tile is one contiguous block:
    # packed[m, p, c, q] = w1g[c*128+p, m*128+q]
    w1_b = np.ascontiguousarray(
        w1g.reshape(DC, P, MC, P).transpose(2, 1, 0, 3).reshape(MC * P, DC * P)
    )
    w2_b = np.ascontiguousarray(w2.astype(bf))

    Q = np.asarray(inputs["Q"], f32)
    K = np.asarray(inputs["K"], f32)
    V = np.asarray(inputs["V"], f32)

    in_maps = []
    for c in range(NCORES):
        b = c // 2
        r0 = (c % 2) * TQ
        m = {
            "q_tok": np.ascontiguousarray(Q[b, r0 : r0 + TQ]),
            "k_tok": np.ascontiguousarray(K[b].astype(bf)),
            "v_tok": np.ascontiguousarray(V[b].astype(bf)),
            "wq": wq_b,
            "wk": wk_b,
            "wv": wv_b,
            "wo": wo_b,
            "w1": w1_b,
            "w2": w2_b,
            "bff1": bff1_tile,
        }
        for name, row in bias_rows.items():
            if np.any(row):
                m["brow_" + name] = row[None, :].astype(bf)
        in_maps.append(m)
    return in_maps, bias_rows


_NC_CACHE = {}


def get_nc(bias_key):
    if bias_key not in _NC_CACHE:
        # bias_key is a tuple of names with nonzero rows; build needs the rows
        # only for zero-checks, so reconstruct flags
        raise KeyError
    return _NC_CACHE[bias_key]


def kernel(**inputs) -> np.ndarray:
    from concourse.bass_utils import run_bass_kernel_spmd

    in_maps, bias_rows = prep_inputs(inputs)
    bias_key = tuple(sorted(n for n, r in bias_rows.items() if np.any(r)))
    if bias_key not in _NC_CACHE:
        _NC_CACHE[bias_key] = build_nc(bias_rows)
    nc = _NC_CACHE[bias_key]
    res = run_bass_kernel_spmd(nc, in_maps, core_ids=list(range(NCORES)))
    out = np.empty((B, N, D), np.float32)
    for c in range(NCORES):
        b = c // 2
        r0 = (c % 2) * TQ
        out[b, r0 : r0 + TQ] = res.results[c]["y"]
    return out



# revision 12
# speedup vs baseline: 1.5679x; 1.5679x over previous
"""CosineTransformerBlock Trainium2 kernel (8 NeuronCores, SPMD).

Sharding: core c handles batch b = c // 2 and query-token rows
[ (c % 2) * 1024 : (c % 2) * 1024 + 1024 ] of that batch.  K/V work is
split by token across the 2 cores sharing a batch; the per-head linear
attention matrices M_h = kn_h^T @ v_h are summed across the pair with a
small HBM AllReduce (262 KB), overlapped with the Q-projection phase.

Key algebraic transforms:
  - cosine attention has no softmax, so (qn @ kn^T) @ v == qn @ (kn^T @ v),
    turning O(N^2) attention into per-head [64,64] matmuls;
  - l2norm is scale-invariant per token-head row, so the LayerNorm
    1/std factor on the Q and K paths vanishes (only the mean is removed);
    V's 1/std is folded into kn's l2 scale (M is bilinear);
  - LN affine g is folded into the following weight matrix on the host,
    and (b @ W) rows are added via K=1 ones-matmuls when nonzero.

Engine/precision strategy:
  - attention path in bf16 (psum f32), FFN matmuls in fp8-e4m3 with
    DoubleRow perf mode (K=256 per instruction);  weights are scaled by
    64 on the host, un-scaled in the gelu / output evacuation;
  - w1/w2 stay resident in SBUF (fp8, 8 MB total) - no streaming;
  - gpsimd (Pool queue) handles SBUF-only elementwise work (it cannot
    touch PSUM); DVE/Act split the PSUM-evacuating ops.
"""

import os
import sys

sys.path.insert(0, "/opt/trn_rl_repo")

import numpy as np
import ml_dtypes

# ---- problem shapes (hardcoded per contract) ----
B, N, D = 4, 2048, 1024
H, DH = 16, 64
INNER = H * DH  # 1024
MLP = 4096
EPS = 1e-5
NCORES = 8
P = 128
TQ = N // 2  # 1024 query tokens per core
DEDUP = bool(int(os.environ.get("BASS_DEDUP", "1")))
TKV = (N // 2) if DEDUP else N  # kv tokens processed per core
DC = D // P  # 8 chunks of the model dim
IC = INNER // P  # 8
MC = MLP // P  # 32
NQT = TQ // P  # 8 q token tiles
NKT = TKV // P  # kv token tiles
GT = 4  # token tiles per FFN group
NG = NQT // GT
W1S = 64.0  # host-side fp8 weight scales
W2S = 64.0

BF16 = None  # set lazily (mybir import)
F32 = None
F8 = None


def _dt():
    global BF16, F32, F8
    import concourse.mybir as mybir

    BF16 = mybir.dt.bfloat16
    F32 = mybir.dt.float32
    F8 = mybir.dt.float8e4
    return mybir


def build_nc(bias_rows, ln1b_nz):
    """Build the SPMD program. bias_rows: dict of host-computed fp32 rows
    (bq, bk, bv, bo, b2: [dim] arrays) - a K=1 ones-matmul is emitted for
    each nonzero row.  ln1b_nz: LN1 beta nonzero -> K/Q paths must apply
    the full (x-mu)*rs standardisation (the rs no longer cancels)."""
    mybir = _dt()
    import concourse.tile as tile
    from concourse import bacc

    AF = mybir.ActivationFunctionType
    ALU = mybir.AluOpType
    PM = mybir.MatmulPerfMode.DoubleRow

    nc = bacc.Bacc("TRN2", target_bir_lowering=False, debug=False, num_devices=NCORES)

    # ---- DRAM I/O ----
    Qd = nc.dram_tensor("q_tok", [TQ, D], F32, kind="ExternalInput").ap()
    Kd = nc.dram_tensor("k_tok", [TKV, D], BF16, kind="ExternalInput").ap()
    Vd = nc.dram_tensor("v_tok", [TKV, D], BF16, kind="ExternalInput").ap()
    wq_d = nc.dram_tensor("wq", [D, INNER], BF16, kind="ExternalInput").ap()
    wk_d = nc.dram_tensor("wk", [D, INNER], BF16, kind="ExternalInput").ap()
    wv_d = nc.dram_tensor("wv", [D, INNER], BF16, kind="ExternalInput").ap()
    wo_d = nc.dram_tensor("wo", [INNER, D], BF16, kind="ExternalInput").ap()
    w1_d = nc.dram_tensor("w1", [P, MC * (DC // 2) * 2 * P], F8, kind="ExternalInput").ap()
    w2_d = nc.dram_tensor("w2", [P, (MC // 2) * 2 * D], F8, kind="ExternalInput").ap()
    bff1_d = nc.dram_tensor("bff1", [P, MC], F32, kind="ExternalInput").ap()
    brow_d = {}
    for name in ("bq", "bk", "bv", "bo", "b2"):
        if np.any(bias_rows[name]):
            brow_d[name] = nc.dram_tensor(
                "brow_" + name, [1, bias_rows[name].shape[0]], BF16,
                kind="ExternalInput",
            ).ap()
    Yd = nc.dram_tensor("y", [TQ, D], F32, kind="ExternalOutput").ap()
    if DEDUP:
        m_part_d = nc.dram_tensor("m_part", [P, IC * DH], F32, kind="Internal").ap()
        m_full_d = nc.dram_tensor("m_full", [P, IC * DH], F32, kind="Internal").ap()

    Qt = Qd.rearrange("(t p) d -> t p d", p=P)
    Kt = Kd.rearrange("(t p) d -> t p d", p=P)
    Vt = Vd.rearrange("(t p) d -> t p d", p=P)
    Yt = Yd.rearrange("(t p) d -> t p d", p=P)
    wq_v = wq_d.rearrange("(c p) n -> p c n", p=P)
    wk_v = wk_d.rearrange("(c p) n -> p c n", p=P)
    wv_v = wv_d.rearrange("(c p) n -> p c n", p=P)
    wo_v = wo_d.rearrange("(c p) n -> p c n", p=P)

    with tile.TileContext(nc) as tc:
        with tc.tile_pool(name="singles", bufs=1) as singles:
            # resident weights (wk/wv live in their own pool, freed after
            # phase 1 so the allocator can reuse the space)
            wq_sb = singles.tile([P, DC, INNER], BF16)
            wo_sb = singles.tile([P, IC, D], BF16)
            w1_sb = singles.tile([P, MC, DC // 2, 2, P], F8)
            w2_sb = singles.tile([P, MC // 2, 2, D], F8)
            bff1_sb = singles.tile([P, MC], F32)
            eps_tile = singles.tile([P, 1], F32)
            ones_row = singles.tile([1, P], BF16)
            nc.vector.memset(eps_tile[:], EPS)
            nc.vector.memset(ones_row[:], 1.0)
            brow_sb = {}
            for name, ap in brow_d.items():
                t = singles.tile([1, ap.shape[1]], BF16, tag="brow_" + name)
                nc.sync.dma_start(t[:], ap[:])
                brow_sb[name] = t
            # M matrices: M_sb[:, pr, :] is blockdiag(M_2pr, M_2pr+1);
            # off-diagonal stays zero.  Mc holds the compact diag blocks.
            M_sb = singles.tile([P, IC, P], BF16)
            nc.vector.memset(M_sb[:], 0.0)
            Mc_sb = singles.tile([P, IC, DH], F32)

            def bias_mm(ps, name, lo, hi, start, stop):
                """Bias row[lo:hi] into psum ps via K=1 matmul (if present).
                Returns (start_consumed, stop_consumed)."""
                if name in brow_sb:
                    nc.tensor.matmul(
                        ps,
                        ones_row[:, : ps.shape[0]],
                        brow_sb[name][:, lo:hi],
                        start=start,
                        stop=stop,
                        skip_group_check=True,
                    )
                    return True
                return False

            def proj(ps_list, xT, w_sb, bname):
                """[tok, 1024] = xT.T @ w (+ bias row) in 2 psum groups."""
                has_b = bname in brow_sb
                for c in range(DC):
                    for g in range(2):
                        nc.tensor.matmul(
                            ps_list[g][:],
                            xT[:, c, :],
                            w_sb[:, c, g * 512 : (g + 1) * 512],
                            start=(c == 0),
                            stop=(c == DC - 1) and not has_b,
                        )
                if has_b:
                    for g in range(2):
                        bias_mm(ps_list[g][:], bname, g * 512, (g + 1) * 512,
                                False, True)

            def l2_rn(pool, mid, pss, tag):
                """Per-head inverse l2 norms from projection psums.
                Returns rn [P, 16, 1] f32 (1/sqrt(sum_h k^2))."""
                ss = pool.tile([P, H, 1], F32, tag=tag + "_ss")
                for g in range(2):
                    sq = mid.tile([P, 512], F32, tag=tag + "_sq")
                    nc.scalar.activation(out=sq[:], in_=pss[g][:], func=AF.Square)
                    nc.vector.reduce_sum(
                        out=ss[:, g * 8 : (g + 1) * 8, :],
                        in_=sq.rearrange("p (h f) -> p h f", h=8),
                        axis=mybir.AxisListType.X,
                    )
                rn = pool.tile([P, H, 1], F32, tag=tag + "_rn")
                nc.scalar.activation(
                    out=rn[:], in_=ss[:], func=AF.Sqrt, scale=1.0
                )
                nc.vector.reciprocal(out=rn[:], in_=rn[:])
                return rn

            # ---------------- Phase 1: K/V -> partial M ----------------
            with (
                tc.tile_pool(name="wkv", bufs=1) as wkv,
                tc.tile_pool(name="kv_io", bufs=4) as kv_io,
                tc.tile_pool(name="kv_mid", bufs=3) as kv_mid,
                tc.tile_pool(name="kv_stats", bufs=4) as kv_stats,
                tc.tile_pool(name="kv_ps", bufs=6, space="PSUM") as kv_ps,
                tc.tile_pool(name="m_ps", bufs=1, space="PSUM") as m_ps_pool,
            ):
                wk_sb = wkv.tile([P, DC, INNER], BF16)
                wv_sb = wkv.tile([P, DC, INNER], BF16)
                nc.gpsimd.dma_start(wk_sb[:], wk_v[:])
                nc.gpsimd.dma_start(wv_sb[:], wv_v[:])
                # later-phase residents: queue the loads behind them
                nc.scalar.dma_start(wq_sb[:], wq_v[:])
                nc.scalar.dma_start(wo_sb[:], wo_v[:])
                nc.scalar.dma_start(
                    w1_sb.rearrange("p a b c d -> p (a b c d)"), w1_d[:]
                )
                nc.gpsimd.dma_start(
                    w2_sb.rearrange("p a b d -> p (a b d)"), w2_d[:]
                )
                nc.sync.dma_start(bff1_sb[:], bff1_d[:])

                M_ps = m_ps_pool.tile([P, IC, P], F32)
                for t in range(NKT):
                    k_in = kv_io.tile([P, D], BF16, tag="k_in")
                    v_in = kv_io.tile([P, D], BF16, tag="v_in")
                    nc.sync.dma_start(k_in[:], Kt[t][:])
                    nc.sync.dma_start(v_in[:], Vt[t][:])
                    # --- LN statistics ---
                    # V always needs mean+var; K needs only the mean unless
                    # LN1 beta is nonzero (then rs no longer cancels).
                    vst = kv_stats.tile([P, 2, 6], F32, tag="v_st")
                    v_v = v_in.rearrange("p (s f) -> p s f", s=2)
                    for sb2 in range(2):
                        nc.vector.bn_stats(
                            out=vst[:, sb2, :], in_=v_v[:, sb2, :]
                        )
                    vmv = kv_stats.tile([P, 1, 2], F32, tag="v_mv")
                    nc.vector.bn_aggr(out=vmv[:], in_=vst[:])
                    v_rs = kv_stats.tile([P, 1], F32, tag="v_rs")
                    # v_rs = 1/sqrt(var + eps)
                    nc.scalar.activation(
                        out=v_rs[:], in_=vmv[:, 0, 1:2], func=AF.Sqrt,
                        bias=eps_tile[:], scale=1.0,
                    )
                    nc.vector.reciprocal(out=v_rs[:], in_=v_rs[:])
                    if ln1b_nz:
                        kst = kv_stats.tile([P, 2, 6], F32, tag="k_st")
                        k_v = k_in.rearrange("p (s f) -> p s f", s=2)
                        for sb2 in range(2):
                            nc.vector.bn_stats(
                                out=kst[:, sb2, :], in_=k_v[:, sb2, :]
                            )
                        kmv = kv_stats.tile([P, 1, 2], F32, tag="k_mv")
                        nc.vector.bn_aggr(out=kmv[:], in_=kst[:])
                        k_rs = kv_stats.tile([P, 1], F32, tag="k_rs")
                        nc.scalar.activation(
                            out=k_rs[:], in_=kmv[:, 0, 1:2], func=AF.Sqrt,
                            bias=eps_tile[:], scale=1.0,
                        )
                        nc.vector.reciprocal(out=k_rs[:], in_=k_rs[:])
                        k_nmu = kv_stats.tile([P, 1], F32, tag="k_nmu")
                        nc.vector.tensor_scalar(
                            out=k_nmu[:], in0=kmv[:, 0, 0:1], scalar1=k_rs[:],
                            scalar2=-1.0, op0=ALU.mult, op1=ALU.mult,
                        )
                        v_nmu = kv_stats.tile([P, 1], F32, tag="v_nmu")
                        nc.vector.tensor_scalar(
                            out=v_nmu[:], in0=vmv[:, 0, 0:1], scalar1=v_rs[:],
                            scalar2=-1.0, op0=ALU.mult, op1=ALU.mult,
                        )
                    else:
                        ksum = kv_stats.tile([P, 1], F32, tag="k_sum")
                        nc.vector.reduce_sum(
                            out=ksum[:], in_=k_in[:], axis=mybir.AxisListType.X
                        )
                        k_nmu = kv_stats.tile([P, 1], F32, tag="k_nmu")
                        nc.gpsimd.tensor_scalar_mul(
                            out=k_nmu[:], in0=ksum[:], scalar1=-1.0 / D
                        )
                        v_nmu = kv_stats.tile([P, 1], F32, tag="v_nmu")
                        nc.gpsimd.tensor_scalar_mul(
                            out=v_nmu[:], in0=vmv[:, 0, 0:1], scalar1=-1.0
                        )
                    # --- centred/standardised activations (SBUF only: Pool) ---
                    kc = kv_mid.tile([P, D], BF16, tag="kc")
                    vc = kv_mid.tile([P, D], BF16, tag="vc")
                    if ln1b_nz:
                        nc.gpsimd.tensor_scalar(
                            out=kc[:], in0=k_in[:], scalar1=k_rs[:],
                            scalar2=k_nmu[:], op0=ALU.mult, op1=ALU.add,
                        )
                        nc.gpsimd.tensor_scalar(
                            out=vc[:], in0=v_in[:], scalar1=v_rs[:],
                            scalar2=v_nmu[:], op0=ALU.mult, op1=ALU.add,
                        )
                    else:
                        nc.gpsimd.tensor_scalar_add(
                            out=kc[:], in0=k_in[:], scalar1=k_nmu[:]
                        )
                        nc.gpsimd.tensor_scalar_add(
                            out=vc[:], in0=v_in[:], scalar1=v_nmu[:]
                        )
                    kT = kv_mid.tile([P, DC, P], BF16, tag="kT")
                    vT = kv_mid.tile([P, DC, P], BF16, tag="vT")
                    for c in range(DC):
                        nc.sync.dma_start(
                            kT[:, c, :], kc[:, c * P : (c + 1) * P], transpose=True
                        )
                        nc.sync.dma_start(
                            vT[:, c, :], vc[:, c * P : (c + 1) * P], transpose=True
                        )
                    # --- projections ---
                    kps = [kv_ps.tile([P, 512], F32, tag="kv_proj", name=f"kps{g}") for g in range(2)]
                    vps = [kv_ps.tile([P, 512], F32, tag="kv_proj", name=f"vps{g}") for g in range(2)]
                    proj(kps, kT, wk_sb, "bk")
                    proj(vps, vT, wv_sb, "bv")
                    # --- kn = k/||k|| * v_rs ; v evacuated as-is ---
                    rnk = l2_rn(kv_stats, kv_mid, kps, "kl2")
                    if not ln1b_nz:
                        # fold V's 1/std into kn (M is bilinear in kn x v)
                        nc.vector.tensor_scalar(
                            out=rnk[:], in0=rnk[:], scalar1=v_rs[:],
                            scalar2=None, op0=ALU.mult,
                        )
                    kn_bf = kv_mid.tile([P, H, DH], BF16, tag="kn_bf")
                    v_bf = kv_mid.tile([P, INNER], BF16, tag="v_bf")
                    for g in range(2):
                        nc.vector.tensor_tensor(
                            out=kn_bf[:, g * 8 : (g + 1) * 8, :],
                            in0=kps[g].rearrange("p (h f) -> p h f", h=8),
                            in1=rnk[:, g * 8 : (g + 1) * 8, :].to_broadcast(
                                [P, 8, DH]
                            ),
                            op=ALU.mult,
                        )
                        nc.scalar.activation(
                            out=v_bf[:, g * 512 : (g + 1) * 512],
                            in_=vps[g][:],
                            func=AF.Copy,
                        )
                    # --- M accumulation (head pairs, blockdiag) ---
                    kn_flat = kn_bf.rearrange("p h f -> p (h f)")
                    for pr in range(IC):
                        nc.tensor.matmul(
                            M_ps[:, pr, :],
                            kn_flat[:, pr * P : (pr + 1) * P],
                            v_bf[:, pr * P : (pr + 1) * P],
                            start=(t == 0 and pr % 4 == 0),
                            stop=(t == NKT - 1 and pr % 4 == 3),
                            skip_group_check=True,
                        )
                # compact diag blocks -> Mc (f32)
                for po in (0, DH):
                    nc.scalar.activation(
                        out=Mc_sb[po : po + DH, :, :],
                        in_=M_ps[po : po + DH, :, po : po + DH],
                        func=AF.Copy,
                    )

            # ---------------- M AllReduce across the batch pair ----------------
            if DEDUP:
                nc.sync.dma_start(
                    m_part_d[:], Mc_sb.rearrange("p c f -> p (c f)")
                )
                nc.gpsimd.collective_compute(
                    "AllReduce",
                    mybir.AluOpType.add,
                    replica_groups=[[0, 1], [2, 3], [4, 5], [6, 7]],
                    ins=[m_part_d[:]],
                    outs=[m_full_d[:]],
                )
                Mf_sb = singles.tile([P, IC, DH], F32)
                nc.sync.dma_start(
                    Mf_sb.rearrange("p c f -> p (c f)"), m_full_d[:]
                )
                for po in (0, DH):
                    nc.gpsimd.tensor_copy(
                        out=M_sb[po : po + DH, :, po : po + DH],
                        in_=Mf_sb[po : po + DH, :, :],
                    )
            else:
                for po in (0, DH):
                    nc.gpsimd.tensor_copy(
                        out=M_sb[po : po + DH, :, po : po + DH],
                        in_=Mc_sb[po : po + DH, :, :],
                    )

            # ---------------- Phase 2: Q -> attn -> x ----------------
            xres = tc.alloc_tile_pool(name="xres", bufs=1)
            x_sb = xres.tile([P, NQT, D], F32)
            with (
                tc.tile_pool(name="q_io", bufs=3) as q_io,
                tc.tile_pool(name="q_mid", bufs=3) as q_mid,
                tc.tile_pool(name="q_stats", bufs=4) as q_stats,
                tc.tile_pool(name="q_ps", bufs=4, space="PSUM") as q_ps,
                tc.tile_pool(name="x_ps", bufs=2, space="PSUM") as x_ps,
                tc.tile_pool(name="at_ps", bufs=1, space="PSUM") as at_ps,
            ):
                for t in range(NQT):
                    q_in = q_io.tile([P, D], F32, tag="q_in")
                    nc.sync.dma_start(q_in[:], Qt[t][:])
                    if ln1b_nz:
                        qst = q_stats.tile([P, 2, 6], F32, tag="q_st")
                        q_v = q_in.rearrange("p (s f) -> p s f", s=2)
                        for sb2 in range(2):
                            nc.vector.bn_stats(
                                out=qst[:, sb2, :], in_=q_v[:, sb2, :]
                            )
                        qmv = q_stats.tile([P, 1, 2], F32, tag="q_mv")
                        nc.vector.bn_aggr(out=qmv[:], in_=qst[:])
                        q_rs = q_stats.tile([P, 1], F32, tag="q_rs")
                        nc.scalar.activation(
                            out=q_rs[:], in_=qmv[:, 0, 1:2], func=AF.Sqrt,
                            bias=eps_tile[:], scale=1.0,
                        )
                        nc.vector.reciprocal(out=q_rs[:], in_=q_rs[:])
                        q_nmu = q_stats.tile([P, 1], F32, tag="q_nmu")
                        nc.vector.tensor_scalar(
                            out=q_nmu[:], in0=qmv[:, 0, 0:1], scalar1=q_rs[:],
                            scalar2=-1.0, op0=ALU.mult, op1=ALU.mult,
                        )
                    else:
                        qsum = q_stats.tile([P, 1], F32, tag="q_sum")
                        nc.vector.reduce_sum(
                            out=qsum[:], in_=q_in[:], axis=mybir.AxisListType.X
                        )
                        q_nmu = q_stats.tile([P, 1], F32, tag="q_nmu")
                        nc.gpsimd.tensor_scalar_mul(
                            out=q_nmu[:], in0=qsum[:], scalar1=-1.0 / D
                        )
                    qc = q_mid.tile([P, D], BF16, tag="qc")
                    if ln1b_nz:
                        nc.gpsimd.tensor_scalar(
                            out=qc[:], in0=q_in[:], scalar1=q_rs[:],
                            scalar2=q_nmu[:], op0=ALU.mult, op1=ALU.add,
                        )
                    else:
                        nc.gpsimd.tensor_scalar_add(
                            out=qc[:], in0=q_in[:], scalar1=q_nmu[:]
                        )
                    qT = q_mid.tile([P, DC, P], BF16, tag="qT")
                    for c in range(DC):
                        nc.sync.dma_start(
                            qT[:, c, :], qc[:, c * P : (c + 1) * P], transpose=True
                        )
                    qps = [q_ps.tile([P, 512], F32, tag="q_ps_t", name=f"qps{g}") for g in range(2)]
                    proj(qps, qT, wq_sb, "bq")
                    rnq = l2_rn(q_stats, q_mid, qps, "ql2")
                    qn_bf = q_mid.tile([P, H, DH], BF16, tag="qn_bf")
                    for g in range(2):
                        nc.vector.tensor_tensor(
                            out=qn_bf[:, g * 8 : (g + 1) * 8, :],
                            in0=qps[g].rearrange("p (h f) -> p h f", h=8),
                            in1=rnq[:, g * 8 : (g + 1) * 8, :].to_broadcast(
                                [P, 8, DH]
                            ),
                            op=ALU.mult,
                        )
                    qn_flat = qn_bf.rearrange("p h f -> p (h f)")
                    qnT = q_mid.tile([P, IC, P], BF16, tag="qnT")
                    for c in range(IC):
                        nc.sync.dma_start(
                            qnT[:, c, :], qn_flat[:, c * P : (c + 1) * P],
                            transpose=True,
                        )
                    # attn^T[h] = M_h^T @ qn_h^T  -> [INNER, tok] chunks
                    a_ps = at_ps.tile([P, IC, P], F32, tag="attn_ps")
                    for pr in range(IC):
                        nc.tensor.matmul(
                            a_ps[:, pr, :],
                            M_sb[:, pr, :],
                            qnT[:, pr, :],
                            start=True,
                            stop=True,
                            skip_group_check=True,
                        )
                    aT_bf = q_mid.tile([P, IC, P], BF16, tag="aT_bf")
                    nc.scalar.activation(out=aT_bf[:], in_=a_ps[:], func=AF.Copy)
                    # x = Q + attn @ wo (+bo)
                    xps = [x_ps.tile([P, 512], F32, tag="x_ps_t", name=f"xps{g}") for g in range(2)]
                    has_bo = "bo" in brow_sb
                    for c in range(IC):
                        for g in range(2):
                            nc.tensor.matmul(
                                xps[g][:],
                                aT_bf[:, c, :],
                                wo_sb[:, c, g * 512 : (g + 1) * 512],
                                start=(c == 0),
                                stop=(c == IC - 1) and not has_bo,
                            )
                    for g in range(2):
                        if has_bo:
                            bias_mm(xps[g][:], "bo", g * 512, (g + 1) * 512,
                                    False, True)
                        nc.vector.tensor_tensor(
                            out=x_sb[:, t, g * 512 : (g + 1) * 512],
                            in0=xps[g][:],
                            in1=q_in[:, g * 512 : (g + 1) * 512],
                            op=ALU.add,
                        )

            # ---------------- Phase 3: FFN (fp8 DoubleRow) ----------------
            with (
                tc.tile_pool(name="f_mid", bufs=2) as f_mid,
                tc.tile_pool(name="f_h", bufs=2) as f_h,
                tc.tile_pool(name="f_stats", bufs=4) as f_stats,
                tc.tile_pool(name="f_out", bufs=3) as f_out,
            ):
                for grp in range(NG):
                    xnT_bf = f_mid.tile([P, DC, GT * P], BF16, tag="xnT_bf")
                    for tt in range(GT):
                        t = grp * GT + tt
                        xst = f_stats.tile([P, 2, 6], F32, tag="x_st")
                        x_v = x_sb[:, t, :].rearrange("p (s f) -> p s f", s=2)
                        for sb2 in range(2):
                            nc.vector.bn_stats(
                                out=xst[:, sb2, :], in_=x_v[:, sb2, :]
                            )
                        xmv = f_stats.tile([P, 1, 2], F32, tag="x_mv")
                        nc.vector.bn_aggr(out=xmv[:], in_=xst[:])
                        x_rs = f_stats.tile([P, 1], F32, tag="x_rs")
                        nc.scalar.activation(
                            out=x_rs[:], in_=xmv[:, 0, 1:2], func=AF.Sqrt,
                            bias=eps_tile[:], scale=1.0,
                        )
                        nc.vector.reciprocal(out=x_rs[:], in_=x_rs[:])
                        x_nmu = f_stats.tile([P, 1], F32, tag="x_nmu")
                        nc.vector.tensor_scalar(
                            out=x_nmu[:], in0=xmv[:, 0, 0:1], scalar1=x_rs[:],
                            scalar2=-1.0, op0=ALU.mult, op1=ALU.mult,
                        )
                        xn = f_mid.tile([P, D], BF16, tag="f_std")
                        nc.gpsimd.tensor_scalar(
                            out=xn[:], in0=x_sb[:, t, :], scalar1=x_rs[:],
                            scalar2=x_nmu[:], op0=ALU.mult, op1=ALU.add,
                        )
                        for c in range(DC):
                            nc.sync.dma_start(
                                xnT_bf[:, c, tt * P : (tt + 1) * P],
                                xn[:, c * P : (c + 1) * P],
                                transpose=True,
                            )
                    xnT_f8 = f_mid.tile([P, DC, GT * P], F8, tag="xnT_f8")
                    for hh in range(2):
                        nc.gpsimd.tensor_copy(
                            out=xnT_f8[:, hh * 4 : (hh + 1) * 4, :],
                            in_=xnT_bf[:, hh * 4 : (hh + 1) * 4, :],
                        )
                    # h^T = gelu((xn @ w1) / W1S + b1) : [MLP, GT*128] fp8
                    h4 = f_h.tile([P, MC, GT * P], F8, tag="h4")
                    with tc.tile_pool(name="h_ps", bufs=3, space="PSUM") as h_ps:
                        for m in range(MC):
                            hp = h_ps.tile([P, GT * P], F32, tag="h_ps_t")
                            for half in range(2):
                                for c in range(DC // 2):
                                    nc.tensor.matmul(
                                        hp[:, half * 256 : (half + 1) * 256],
                                        w1_sb[:, m, c],
                                        xnT_f8[:, 2 * c : 2 * c + 2,
                                               half * 256 : (half + 1) * 256],
                                        start=(half == 0 and c == 0),
                                        stop=(half == 1 and c == DC // 2 - 1),
                                        perf_mode=PM,
                                        skip_group_check=(half == 1),
                                    )
                            nc.scalar.activation(
                                out=h4[:, m, :],
                                in_=hp[:],
                                func=AF.Gelu,
                                bias=bff1_sb[:, m : m + 1],
                                scale=1.0 / W1S,
                            )
                    # y = x + (h @ w2) / W2S (+ b2)
                    has_b2 = "b2" in brow_sb
                    with tc.tile_pool(name="y_ps", bufs=8, space="PSUM") as y_ps:
                        yps = [
                            [
                                y_ps.tile([P, 2, 256], F32, tag="y_ps_t",
                                          name=f"yps_{grp}_{tt}_{bk}")
                                for bk in range(2)
                            ]
                            for tt in range(GT)
                        ]
                        for j in range(MC // 2):
                            for tt in range(GT):
                                for cq in range(4):
                                    nc.tensor.matmul(
                                        yps[tt][cq // 2][:, cq % 2, :],
                                        h4[:, 2 * j : 2 * j + 2,
                                           tt * P : (tt + 1) * P],
                                        w2_sb[:, j, :, cq * 256 : (cq + 1) * 256],
                                        start=(j == 0 and cq % 2 == 0),
                                        stop=(j == MC // 2 - 1 and cq % 2 == 1)
                                        and not has_b2,
                                        perf_mode=PM,
                                        skip_group_check=(cq % 2 == 1),
                                    )
                        for tt in range(GT):
                            t = grp * GT + tt
                            y_out = f_out.tile([P, D], F32, tag="y_out")
                            for cq in range(4):
                                if has_b2 and j == MC // 2 - 1:
                                    pass
                                nc.vector.scalar_tensor_tensor(
                                    out=y_out[:, cq * 256 : (cq + 1) * 256],
                                    in0=yps[tt][cq // 2][:, cq % 2, :],
                                    scalar=1.0 / W2S,
                                    in1=x_sb[:, t, cq * 256 : (cq + 1) * 256],
                                    op0=ALU.mult,
                                    op1=ALU.add,
                                )
                            nc.sync.dma_start(Yt[t][:], y_out[:])
                    if has_b2:
                        raise NotImplementedError(
                            "nonzero b2 needs the bias row folded in; "
                            "scale b2 by W2S on the host and emit bias_mm "
                            "with stop=True before the evacuation"
                        )
            xres.release()

    nc.compile()
    return nc


def prep_inputs(inputs):
    """Host-side shard + weight folding. Returns (in_maps, bias_rows, ln1b_nz)."""
    f32 = np.float32
    bf = ml_dtypes.bfloat16
    f8 = ml_dtypes.float8_e4m3
    g1 = np.asarray(inputs["ln1_g"], f32)
    b1ln = np.asarray(inputs["ln1_b"], f32)
    g2 = np.asarray(inputs["ln2_g"], f32)
    b2ln = np.asarray(inputs["ln2_b"], f32)
    wq = np.asarray(inputs["wq"], f32)
    wk = np.asarray(inputs["wk"], f32)
    wv = np.asarray(inputs["wv"], f32)
    wo = np.asarray(inputs["wo"], f32)
    w1 = np.asarray(inputs["w1"], f32)
    w2 = np.asarray(inputs["w2"], f32)

    bias_rows = {
        "bq": (b1ln @ wq).astype(f32),
        "bk": (b1ln @ wk).astype(f32),
        "bv": (b1ln @ wv).astype(f32),
        "bo": np.asarray(inputs["bo"], f32),
        "b2": np.asarray(inputs["b2"], f32),
    }
    ln1b_nz = bool(np.any(b1ln))
    bff1 = (b2ln @ w1 + np.asarray(inputs["b1"], f32)).astype(f32)
    bff1_tile = np.ascontiguousarray(bff1.reshape(MC, P).T)  # [P, MC]

    wq_b = np.ascontiguousarray((g1[:, None] * wq).astype(bf))
    wk_b = np.ascontiguousarray((g1[:, None] * wk).astype(bf))
    wv_b = np.ascontiguousarray((g1[:, None] * wv).astype(bf))
    wo_b = np.ascontiguousarray(wo.astype(bf))
    # fp8 FFN weights, pre-scaled; packed for DoubleRow operands:
    #   w1p[p, m, c, i, q] = (g2*w1*W1S)[(2c+i)*128+p, m*128+q]  (lhsT)
    #   w2p[p, j, i, n]    = (w2*W2S)[(2j+i)*128+p, n]           (rhs)
    w1g = (g2[:, None] * w1 * W1S).astype(f8)
    w1_b = np.ascontiguousarray(
        w1g.reshape(DC // 2, 2, P, MC, P)
        .transpose(2, 3, 0, 1, 4)
        .reshape(P, MC * (DC // 2) * 2 * P)
    )
    w2s = (w2 * W2S).astype(f8)
    w2_b = np.ascontiguousarray(
        w2s.reshape(MC // 2, 2, P, D).transpose(2, 0, 1, 3).reshape(P, (MC // 2) * 2 * D)
    )

    Q = np.asarray(inputs["Q"], f32)
    K = np.asarray(inputs["K"], f32)
    V = np.asarray(inputs["V"], f32)

    in_maps = []
    for c in range(NCORES):
        b = c // 2
        r0 = (c % 2) * TQ
        if DEDUP:
            kslc = slice(r0, r0 + TKV)
        else:
            kslc = slice(0, N)
        m = {
            "q_tok": np.ascontiguousarray(Q[b, r0 : r0 + TQ]),
            "k_tok": np.ascontiguousarray(K[b, kslc].astype(bf)),
            "v_tok": np.ascontiguousarray(V[b, kslc].astype(bf)),
            "wq": wq_b,
            "wk": wk_b,
            "wv": wv_b,
            "wo": wo_b,
            "w1": w1_b,
            "w2": w2_b,
            "bff1": bff1_tile,
        }
        for name, row in bias_rows.items():
            if np.any(row):
                m["brow_" + name] = row[None, :].astype(bf)
        in_maps.append(m)
    return in_maps, bias_rows, ln1b_nz


_NC_CACHE = {}


def kernel(**inputs) -> np.ndarray:
    from concourse.bass_utils import run_bass_kernel_spmd

    in_maps, bias_rows, ln1b_nz = prep_inputs(inputs)
    bias_key = (ln1b_nz,) + tuple(
        sorted(n for n, r in bias_rows.items() if np.any(r))
    )
    if bias_key not in _NC_CACHE:
        _NC_CACHE[bias_key] = build_nc(bias_rows, ln1b_nz)
    nc = _NC_CACHE[bias_key]
    res = run_bass_kernel_spmd(nc, in_maps, core_ids=list(range(NCORES)))
    out = np.empty((B, N, D), np.float32)
    for c in range(NCORES):
        b = c // 2
        r0 = (c % 2) * TQ
        out[b, r0 : r0 + TQ] = res.results[c]["y"]
    return out


# revision 13
# speedup vs baseline: 1.6398x; 1.0459x over previous
"""CosineTransformerBlock Trainium2 kernel (8 NeuronCores, SPMD).

Sharding: core c handles batch b = c // 2 and query-token rows
[ (c % 2) * 1024 : (c % 2) * 1024 + 1024 ] of that batch.  K/V work is
split by token across the 2 cores sharing a batch; the per-head linear
attention matrices M_h = kn_h^T @ v_h are summed across the pair with a
small HBM AllReduce (128 KB bf16), overlapped with the Q-projection
phase (which is emitted before anything that reads M so no engine queue
head-of-line blocks on the collective).

Key algebraic transforms:
  - cosine attention has no softmax, so (qn @ kn^T) @ v == qn @ (kn^T @ v),
    turning O(N^2) attention into per-head [64,64] matmuls;
  - l2norm is scale-invariant per token-head row, so the LayerNorm
    1/std factor on the Q and K paths vanishes (only the mean is removed);
    V's 1/std is folded into kn's l2 scale (M is bilinear);
  - LN affine g is folded into the following weight matrix on the host,
    and (b @ W) rows are added via K=1 ones-matmuls when nonzero.

Engine/precision strategy:
  - attention path in bf16 (psum f32), FFN matmuls in fp8-e4m3 with
    DoubleRow perf mode (K=256 per instruction); weights are scaled by
    64 on the host, un-scaled in the gelu / output evacuation;
  - w1/w2 stay resident in SBUF (fp8, 8 MB total), loaded on the Pool
    queue during phase 1 (Pool runs nothing else then);
  - gpsimd/Pool cannot touch PSUM and sits behind the collective, so
    PSUM evacuations and phase-2a elementwise work live on DVE/Act.
"""

import os
import sys

sys.path.insert(0, "/opt/trn_rl_repo")

import numpy as np
import ml_dtypes

# ---- problem shapes (hardcoded per contract) ----
B, N, D = 4, 2048, 1024
H, DH = 16, 64
INNER = H * DH  # 1024
MLP = 4096
EPS = 1e-5
NCORES = 8
P = 128
TQ = N // 2  # 1024 query tokens per core
DEDUP = bool(int(os.environ.get("BASS_DEDUP", "1")))
TKV = (N // 2) if DEDUP else N  # kv tokens processed per core
DC = D // P  # 8 chunks of the model dim
IC = INNER // P  # 8
MC = MLP // P  # 32
NQT = TQ // P  # 8 q token tiles
NKT = TKV // P  # kv token tiles
GT = 4  # token tiles per FFN group
NG = NQT // GT
W1S = 64.0  # host-side fp8 weight scales
W2S = 64.0

BF16 = None  # set lazily (mybir import)
F32 = None
F8 = None


def _dt():
    global BF16, F32, F8
    import concourse.mybir as mybir

    BF16 = mybir.dt.bfloat16
    F32 = mybir.dt.float32
    F8 = mybir.dt.float8e4
    return mybir


def build_nc(bias_rows, ln1b_nz):
    """Build the SPMD program. bias_rows: dict of host-computed fp32 rows
    (bq, bk, bv, bo, b2: [dim] arrays) - a K=1 ones-matmul is emitted for
    each nonzero row.  ln1b_nz: LN1 beta nonzero -> K/Q paths must apply
    the full (x-mu)*rs standardisation (the rs no longer cancels)."""
    mybir = _dt()
    import concourse.tile as tile
    from concourse import bacc

    AF = mybir.ActivationFunctionType
    ALU = mybir.AluOpType
    PM = mybir.MatmulPerfMode.DoubleRow

    nc = bacc.Bacc("TRN2", target_bir_lowering=False, debug=False, num_devices=NCORES)

    # ---- DRAM I/O ----
    Qd = nc.dram_tensor("q_tok", [TQ, D], F32, kind="ExternalInput").ap()
    Kd = nc.dram_tensor("k_tok", [TKV, D], BF16, kind="ExternalInput").ap()
    Vd = nc.dram_tensor("v_tok", [TKV, D], BF16, kind="ExternalInput").ap()
    wq_d = nc.dram_tensor("wq", [D, INNER], BF16, kind="ExternalInput").ap()
    wk_d = nc.dram_tensor("wk", [D, INNER], BF16, kind="ExternalInput").ap()
    wv_d = nc.dram_tensor("wv", [D, INNER], BF16, kind="ExternalInput").ap()
    wo_d = nc.dram_tensor("wo", [INNER, D], BF16, kind="ExternalInput").ap()
    w1_d = nc.dram_tensor("w1", [P, MC * (DC // 2) * 2 * P], F8, kind="ExternalInput").ap()
    w2_d = nc.dram_tensor("w2", [P, (MC // 2) * 2 * D], F8, kind="ExternalInput").ap()
    bff1_d = nc.dram_tensor("bff1", [P, MC], F32, kind="ExternalInput").ap()
    brow_d = {}
    for name in ("bq", "bk", "bv", "bo", "b2"):
        if np.any(bias_rows[name]):
            brow_d[name] = nc.dram_tensor(
                "brow_" + name, [1, bias_rows[name].shape[0]], BF16,
                kind="ExternalInput",
            ).ap()
    Yd = nc.dram_tensor("y", [TQ, D], F32, kind="ExternalOutput").ap()
    if DEDUP:
        m_part_d = nc.dram_tensor("m_part", [P, IC * DH], BF16, kind="Internal").ap()
        m_full_d = nc.dram_tensor("m_full", [P, IC * DH], BF16, kind="Internal").ap()

    Qt = Qd.rearrange("(t p) d -> t p d", p=P)
    Kt = Kd.rearrange("(t p) d -> t p d", p=P)
    Vt = Vd.rearrange("(t p) d -> t p d", p=P)
    Yt = Yd.rearrange("(t p) d -> t p d", p=P)
    wq_v = wq_d.rearrange("(c p) n -> p c n", p=P)
    wk_v = wk_d.rearrange("(c p) n -> p c n", p=P)
    wv_v = wv_d.rearrange("(c p) n -> p c n", p=P)
    wo_v = wo_d.rearrange("(c p) n -> p c n", p=P)

    with tile.TileContext(nc) as tc:
        with tc.tile_pool(name="singles", bufs=1) as singles:
            wq_sb = singles.tile([P, DC, INNER], BF16)
            wo_sb = singles.tile([P, IC, D], BF16)
            w1_sb = singles.tile([P, MC, DC // 2, 2, P], F8)
            w2_sb = singles.tile([P, MC // 2, 2, D], F8)
            bff1_sb = singles.tile([P, MC], F32)
            eps_tile = singles.tile([P, 1], F32)
            ones_row = singles.tile([1, P], BF16)
            nc.vector.memset(eps_tile[:], EPS)
            nc.vector.memset(ones_row[:], 1.0)
            brow_sb = {}
            for name, ap in brow_d.items():
                t = singles.tile([1, ap.shape[1]], BF16, tag="brow_" + name)
                nc.sync.dma_start(t[:], ap[:])
                brow_sb[name] = t
            # M matrices: M_sb[:, pr, :] is blockdiag(M_2pr, M_2pr+1);
            # off-diagonal stays zero.  Mc holds the compact diag blocks.
            M_sb = singles.tile([P, IC, P], BF16)
            nc.vector.memset(M_sb[:], 0.0)
            Mc_sb = singles.tile([P, IC, DH], BF16)

            def bias_mm(ps, name, lo, hi, start, stop):
                if name in brow_sb:
                    nc.tensor.matmul(
                        ps,
                        ones_row[:, : ps.shape[0]],
                        brow_sb[name][:, lo:hi],
                        start=start,
                        stop=stop,
                        skip_group_check=True,
                    )
                    return True
                return False

            def proj(ps_list, xT, w_sb, bname):
                """[tok, 1024] = xT.T @ w (+ bias row) in 2 psum groups."""
                has_b = bname in brow_sb
                for c in range(DC):
                    for g in range(2):
                        nc.tensor.matmul(
                            ps_list[g][:],
                            xT[:, c, :],
                            w_sb[:, c, g * 512 : (g + 1) * 512],
                            start=(c == 0),
                            stop=(c == DC - 1) and not has_b,
                        )
                if has_b:
                    for g in range(2):
                        bias_mm(ps_list[g][:], bname, g * 512, (g + 1) * 512,
                                False, True)

            def l2_rn(pool, mid, pss, tag):
                """Per-head inverse l2 norms from projection psums.
                Returns rn [P, 16, 1] f32."""
                ss = pool.tile([P, H, 1], F32, tag=tag + "_ss")
                for g in range(2):
                    sq = mid.tile([P, 512], F32, tag=tag + "_sq")
                    nc.scalar.activation(out=sq[:], in_=pss[g][:], func=AF.Square)
                    nc.vector.reduce_sum(
                        out=ss[:, g * 8 : (g + 1) * 8, :],
                        in_=sq.rearrange("p (h f) -> p h f", h=8),
                        axis=mybir.AxisListType.X,
                    )
                rn = pool.tile([P, H, 1], F32, tag=tag + "_rn")
                nc.scalar.activation(
                    out=rn[:], in_=ss[:], func=AF.Sqrt, scale=1.0
                )
                nc.vector.reciprocal(out=rn[:], in_=rn[:])
                return rn

            def center_stats(pool, x_in, tag, need_var):
                """LN1 stats on DVE.  Returns (nmu, rs) [P,1] f32 tiles
                (rs is None unless need_var)."""
                if need_var:
                    st = pool.tile([P, 2, 6], F32, tag=tag + "_st")
                    x_v = x_in.rearrange("p (s f) -> p s f", s=2)
                    for sb2 in range(2):
                        nc.vector.bn_stats(out=st[:, sb2, :], in_=x_v[:, sb2, :])
                    mv = pool.tile([P, 1, 2], F32, tag=tag + "_mv")
                    nc.vector.bn_aggr(out=mv[:], in_=st[:])
                    rs = pool.tile([P, 1], F32, tag=tag + "_rs")
                    nc.scalar.activation(
                        out=rs[:], in_=mv[:, 0, 1:2], func=AF.Sqrt,
                        bias=eps_tile[:], scale=1.0,
                    )
                    nc.vector.reciprocal(out=rs[:], in_=rs[:])
                    nmu = pool.tile([P, 1], F32, tag=tag + "_nmu")
                    if ln1b_nz:
                        # nmu = -mu * rs  (full std path)
                        nc.vector.tensor_scalar(
                            out=nmu[:], in0=mv[:, 0, 0:1], scalar1=rs[:],
                            scalar2=-1.0, op0=ALU.mult, op1=ALU.mult,
                        )
                    else:
                        nc.vector.tensor_scalar_mul(
                            out=nmu[:], in0=mv[:, 0, 0:1], scalar1=-1.0
                        )
                    return nmu, rs
                sm = pool.tile([P, 1], F32, tag=tag + "_sum")
                nc.vector.reduce_sum(
                    out=sm[:], in_=x_in, axis=mybir.AxisListType.X
                )
                nmu = pool.tile([P, 1], F32, tag=tag + "_nmu")
                nc.vector.tensor_scalar_mul(out=nmu[:], in0=sm[:], scalar1=-1.0 / D)
                return nmu, None

            def center_apply(out_t, x_in, nmu, rs):
                """xc = x - mu  (or (x-mu)*rs when LN1 beta nonzero): Act."""
                if ln1b_nz:
                    nc.scalar.activation(
                        out=out_t, in_=x_in, func=AF.Identity,
                        bias=nmu, scale=rs,
                    )
                else:
                    nc.scalar.activation(
                        out=out_t, in_=x_in, func=AF.Identity,
                        bias=nmu, scale=1.0,
                    )

            # ---------------- Phase 1: K/V -> partial M ----------------
            with (
                tc.tile_pool(name="wkv", bufs=1) as wkv,
                tc.tile_pool(name="kv_io", bufs=4) as kv_io,
                tc.tile_pool(name="kv_mid", bufs=3) as kv_mid,
                tc.tile_pool(name="kv_stats", bufs=4) as kv_stats,
                tc.tile_pool(name="kv_ps", bufs=6, space="PSUM") as kv_ps,
                tc.tile_pool(name="m_ps", bufs=1, space="PSUM") as m_ps_pool,
            ):
                # Pool queue carries ONLY these loads during phase 1
                wk_sb = wkv.tile([P, DC, INNER], BF16)
                wv_sb = wkv.tile([P, DC, INNER], BF16)
                nc.gpsimd.dma_start(wk_sb[:], wk_v[:])
                nc.gpsimd.dma_start(wv_sb[:], wv_v[:])
                nc.gpsimd.dma_start(
                    w1_sb.rearrange("p a b c d -> p (a b c d)"), w1_d[:]
                )
                nc.gpsimd.dma_start(
                    w2_sb.rearrange("p a b d -> p (a b d)"), w2_d[:]
                )
                nc.sync.dma_start(bff1_sb[:], bff1_d[:])

                M_ps = m_ps_pool.tile([P, IC, P], F32)
                for t in range(NKT):
                    k_in = kv_io.tile([P, D], BF16, tag="k_in")
                    v_in = kv_io.tile([P, D], BF16, tag="v_in")
                    nc.sync.dma_start(k_in[:], Kt[t][:])
                    nc.sync.dma_start(v_in[:], Vt[t][:])
                    if t == 1:
                        # phase-2 residents: emit behind tile-0 work so the
                        # Act queue isn't clogged at startup
                        nc.scalar.dma_start(wq_sb[:], wq_v[:])
                    if t == 2:
                        nc.scalar.dma_start(wo_sb[:], wo_v[:])
                    k_nmu, k_rs = center_stats(kv_stats, k_in[:], "k", ln1b_nz)
                    v_nmu, v_rs = center_stats(kv_stats, v_in[:], "v", True)
                    kc = kv_mid.tile([P, D], BF16, tag="kc")
                    vc = kv_mid.tile([P, D], BF16, tag="vc")
                    center_apply(kc[:], k_in[:], k_nmu[:], k_rs[:] if k_rs else None)
                    center_apply(vc[:], v_in[:], v_nmu[:], v_rs[:] if ln1b_nz else None)
                    kT = kv_mid.tile([P, DC, P], BF16, tag="kT")
                    vT = kv_mid.tile([P, DC, P], BF16, tag="vT")
                    for c in range(DC):
                        nc.sync.dma_start(
                            kT[:, c, :], kc[:, c * P : (c + 1) * P], transpose=True
                        )
                        nc.sync.dma_start(
                            vT[:, c, :], vc[:, c * P : (c + 1) * P], transpose=True
                        )
                    kps = [kv_ps.tile([P, 512], F32, tag="kv_proj", name=f"kps{g}")
                           for g in range(2)]
                    vps = [kv_ps.tile([P, 512], F32, tag="kv_proj", name=f"vps{g}")
                           for g in range(2)]
                    proj(kps, kT, wk_sb, "bk")
                    proj(vps, vT, wv_sb, "bv")
                    rnk = l2_rn(kv_stats, kv_mid, kps, "kl2")
                    if not ln1b_nz:
                        # fold V's 1/std into kn (M is bilinear in kn x v)
                        nc.vector.tensor_scalar(
                            out=rnk[:], in0=rnk[:], scalar1=v_rs[:],
                            scalar2=None, op0=ALU.mult,
                        )
                    kn_bf = kv_mid.tile([P, H, DH], BF16, tag="kn_bf")
                    v_bf = kv_mid.tile([P, INNER], BF16, tag="v_bf")
                    for g in range(2):
                        nc.vector.tensor_tensor(
                            out=kn_bf[:, g * 8 : (g + 1) * 8, :],
                            in0=kps[g].rearrange("p (h f) -> p h f", h=8),
                            in1=rnk[:, g * 8 : (g + 1) * 8, :].to_broadcast(
                                [P, 8, DH]
                            ),
                            op=ALU.mult,
                        )
                        nc.scalar.activation(
                            out=v_bf[:, g * 512 : (g + 1) * 512],
                            in_=vps[g][:],
                            func=AF.Copy,
                        )
                    kn_flat = kn_bf.rearrange("p h f -> p (h f)")
                    for pr in range(IC):
                        nc.tensor.matmul(
                            M_ps[:, pr, :],
                            kn_flat[:, pr * P : (pr + 1) * P],
                            v_bf[:, pr * P : (pr + 1) * P],
                            start=(t == 0 and pr % 4 == 0),
                            stop=(t == NKT - 1 and pr % 4 == 3),
                            skip_group_check=True,
                        )
                # compact diag blocks -> Mc (bf16)
                for po in (0, DH):
                    nc.scalar.activation(
                        out=Mc_sb[po : po + DH, :, :],
                        in_=M_ps[po : po + DH, :, po : po + DH],
                        func=AF.Copy,
                    )

            if DEDUP:
                # ship partial M; the collective runs on the Pool queue while
                # phase 2a (emitted next, M-independent) keeps PE/DVE/SP busy
                nc.sync.dma_start(
                    m_part_d[:], Mc_sb.rearrange("p c f -> p (c f)")
                )
                nc.gpsimd.collective_compute(
                    "AllReduce",
                    mybir.AluOpType.add,
                    replica_groups=[[0, 1], [2, 3], [4, 5], [6, 7]],
                    ins=[m_part_d[:]],
                    outs=[m_full_d[:]],
                )
            else:
                for po in (0, DH):
                    nc.gpsimd.tensor_copy(
                        out=M_sb[po : po + DH, :, po : po + DH],
                        in_=Mc_sb[po : po + DH, :, :],
                    )

            # ---------------- Phase 2a: Q projections (M-independent) -------
            xres = tc.alloc_tile_pool(name="xres", bufs=1)
            x_sb = xres.tile([P, NQT, D], F32)
            qn_pool = tc.alloc_tile_pool(name="qn_all", bufs=1)
            qnT_all = qn_pool.tile([P, NQT, IC, P], BF16)
            with (
                tc.tile_pool(name="q_mid", bufs=3) as q_mid,
                tc.tile_pool(name="q_stats", bufs=4) as q_stats,
                tc.tile_pool(name="q_ps", bufs=4, space="PSUM") as q_ps,
            ):
                for t in range(NQT):
                    q_in = x_sb[:, t, :]
                    nc.sync.dma_start(q_in, Qt[t][:])
                    q_nmu, q_rs = center_stats(q_stats, q_in, "q", ln1b_nz)
                    qc = q_mid.tile([P, D], BF16, tag="qc")
                    center_apply(qc[:], q_in, q_nmu[:], q_rs[:] if q_rs else None)
                    qT = q_mid.tile([P, DC, P], BF16, tag="qT")
                    for c in range(DC):
                        nc.sync.dma_start(
                            qT[:, c, :], qc[:, c * P : (c + 1) * P], transpose=True
                        )
                    qps = [q_ps.tile([P, 512], F32, tag="q_ps_t", name=f"qps{g}")
                           for g in range(2)]
                    proj(qps, qT, wq_sb, "bq")
                    rnq = l2_rn(q_stats, q_mid, qps, "ql2")
                    qn_bf = q_mid.tile([P, H, DH], BF16, tag="qn_bf")
                    for g in range(2):
                        nc.vector.tensor_tensor(
                            out=qn_bf[:, g * 8 : (g + 1) * 8, :],
                            in0=qps[g].rearrange("p (h f) -> p h f", h=8),
                            in1=rnq[:, g * 8 : (g + 1) * 8, :].to_broadcast(
                                [P, 8, DH]
                            ),
                            op=ALU.mult,
                        )
                    qn_flat = qn_bf.rearrange("p h f -> p (h f)")
                    for c in range(IC):
                        nc.sync.dma_start(
                            qnT_all[:, t, c, :], qn_flat[:, c * P : (c + 1) * P],
                            transpose=True,
                        )

            # ---------------- Phase 2b: attn + wo + residual ----------------
            if DEDUP:
                Mf_sb = singles.tile([P, IC, DH], BF16)
                nc.scalar.dma_start(
                    Mf_sb.rearrange("p c f -> p (c f)"), m_full_d[:]
                )
                for po in (0, DH):
                    nc.gpsimd.tensor_copy(
                        out=M_sb[po : po + DH, :, po : po + DH],
                        in_=Mf_sb[po : po + DH, :, :],
                    )
            with (
                tc.tile_pool(name="a_mid", bufs=3) as a_mid,
                tc.tile_pool(name="at_ps", bufs=2, space="PSUM") as at_ps,
                tc.tile_pool(name="x_ps", bufs=4, space="PSUM") as x_ps,
            ):
                for t in range(NQT):
                    a_ps = at_ps.tile([P, IC, P], F32, tag="attn_ps")
                    for pr in range(IC):
                        nc.tensor.matmul(
                            a_ps[:, pr, :],
                            M_sb[:, pr, :],
                            qnT_all[:, t, pr, :],
                            start=True,
                            stop=True,
                            skip_group_check=True,
                        )
                    aT_bf = a_mid.tile([P, IC, P], BF16, tag="aT_bf")
                    nc.scalar.activation(out=aT_bf[:], in_=a_ps[:], func=AF.Copy)
                    # x = Q + attn @ wo (+bo), residual in place
                    xps = [x_ps.tile([P, 512], F32, tag="x_ps_t", name=f"xps{g}")
                           for g in range(2)]
                    has_bo = "bo" in brow_sb
                    for c in range(IC):
                        for g in range(2):
                            nc.tensor.matmul(
                                xps[g][:],
                                aT_bf[:, c, :],
                                wo_sb[:, c, g * 512 : (g + 1) * 512],
                                start=(c == 0),
                                stop=(c == IC - 1) and not has_bo,
                            )
                    for g in range(2):
                        if has_bo:
                            bias_mm(xps[g][:], "bo", g * 512, (g + 1) * 512,
                                    False, True)
                        nc.vector.tensor_tensor(
                            out=x_sb[:, t, g * 512 : (g + 1) * 512],
                            in0=xps[g][:],
                            in1=x_sb[:, t, g * 512 : (g + 1) * 512],
                            op=ALU.add,
                        )
            qn_pool.release()

            # ---------------- Phase 3: FFN (fp8 DoubleRow) ----------------
            with (
                tc.tile_pool(name="f_mid", bufs=2) as f_mid,
                tc.tile_pool(name="f_h", bufs=2) as f_h,
                tc.tile_pool(name="f_stats", bufs=4) as f_stats,
                tc.tile_pool(name="f_out", bufs=3) as f_out,
            ):
                # hoist the xn^T prep for ALL groups so it pipelines behind
                # phase 2b instead of serialising between the matmul loops
                xnT_f8s = []
                for grp in range(NG):
                    xnT_bf = f_mid.tile([P, DC, GT * P], BF16, tag="xnT_bf",
                                        name=f"xnTb{grp}")
                    for tt in range(GT):
                        t = grp * GT + tt
                        xst = f_stats.tile([P, 2, 6], F32, tag="x_st")
                        x_v = x_sb[:, t, :].rearrange("p (s f) -> p s f", s=2)
                        for sb2 in range(2):
                            nc.vector.bn_stats(
                                out=xst[:, sb2, :], in_=x_v[:, sb2, :]
                            )
                        xmv = f_stats.tile([P, 1, 2], F32, tag="x_mv")
                        nc.vector.bn_aggr(out=xmv[:], in_=xst[:])
                        x_rs = f_stats.tile([P, 1], F32, tag="x_rs")
                        nc.scalar.activation(
                            out=x_rs[:], in_=xmv[:, 0, 1:2], func=AF.Sqrt,
                            bias=eps_tile[:], scale=1.0,
                        )
                        nc.vector.reciprocal(out=x_rs[:], in_=x_rs[:])
                        x_nmu = f_stats.tile([P, 1], F32, tag="x_nmu")
                        nc.vector.tensor_scalar(
                            out=x_nmu[:], in0=xmv[:, 0, 0:1], scalar1=x_rs[:],
                            scalar2=-1.0, op0=ALU.mult, op1=ALU.mult,
                        )
                        xn = f_mid.tile([P, D], BF16, tag="f_std")
                        nc.scalar.activation(
                            out=xn[:], in_=x_sb[:, t, :], func=AF.Identity,
                            bias=x_nmu[:], scale=x_rs[:],
                        )
                        for c in range(DC):
                            nc.sync.dma_start(
                                xnT_bf[:, c, tt * P : (tt + 1) * P],
                                xn[:, c * P : (c + 1) * P],
                                transpose=True,
                            )
                    xnT_f8 = f_mid.tile([P, DC, GT * P], F8, tag="xnT_f8",
                                        name=f"xnTf{grp}")
                    for hh in range(2):
                        nc.gpsimd.tensor_copy(
                            out=xnT_f8[:, hh * 4 : (hh + 1) * 4, :],
                            in_=xnT_bf[:, hh * 4 : (hh + 1) * 4, :],
                        )
                    xnT_f8s.append(xnT_f8)

                if "b2" in brow_sb:
                    raise NotImplementedError(
                        "nonzero b2: scale the row by W2S on the host and "
                        "append a bias_mm to each y psum chain"
                    )
                for grp in range(NG):
                    xnT_f8 = xnT_f8s[grp]
                    # h^T = gelu((xn @ w1) / W1S + b1) : [MLP, GT*128] fp8
                    h4 = f_h.tile([P, MC, GT * P], F8, tag="h4")
                    with tc.tile_pool(name="h_ps", bufs=2, space="PSUM") as h_ps:
                        for m in range(MC):
                            hp = h_ps.tile([P, GT * P], F32, tag="h_ps_t")
                            for half in range(2):
                                for c in range(DC // 2):
                                    nc.tensor.matmul(
                                        hp[:, half * 256 : (half + 1) * 256],
                                        w1_sb[:, m, c],
                                        xnT_f8[:, 2 * c : 2 * c + 2,
                                               half * 256 : (half + 1) * 256],
                                        start=(half == 0 and c == 0),
                                        stop=(half == 1 and c == DC // 2 - 1),
                                        perf_mode=PM,
                                        skip_group_check=(half == 1),
                                    )
                            nc.scalar.activation(
                                out=h4[:, m, :],
                                in_=hp[:],
                                func=AF.Gelu,
                                bias=bff1_sb[:, m : m + 1],
                                scale=1.0 / W1S,
                            )
                    # y = x + (h @ w2) / W2S, one token tile at a time so the
                    # evacuation/DMA of tile tt overlaps tile tt+1's matmuls
                    with tc.tile_pool(name="y_ps", bufs=3, space="PSUM") as y_ps:
                        for tt in range(GT):
                            t = grp * GT + tt
                            yp = y_ps.tile([P, 4, 256], F32, tag="y_ps_t")
                            for j in range(MC // 2):
                                for cq in range(4):
                                    nc.tensor.matmul(
                                        yp[:, cq, :],
                                        h4[:, 2 * j : 2 * j + 2,
                                           tt * P : (tt + 1) * P],
                                        w2_sb[:, j, :, cq * 256 : (cq + 1) * 256],
                                        start=(j == 0 and cq % 2 == 0),
                                        stop=(j == MC // 2 - 1 and cq % 2 == 1),
                                        perf_mode=PM,
                                        skip_group_check=(cq % 2 == 1),
                                    )
                            y_out = f_out.tile([P, D], F32, tag="y_out")
                            for cq in range(4):
                                nc.vector.scalar_tensor_tensor(
                                    out=y_out[:, cq * 256 : (cq + 1) * 256],
                                    in0=yp[:, cq, :],
                                    scalar=1.0 / W2S,
                                    in1=x_sb[:, t, cq * 256 : (cq + 1) * 256],
                                    op0=ALU.mult,
                                    op1=ALU.add,
                                )
                            nc.sync.dma_start(Yt[t][:], y_out[:])
            xres.release()

    nc.compile()
    return nc


def prep_inputs(inputs):
    """Host-side shard + weight folding. Returns (in_maps, bias_rows, ln1b_nz)."""
    f32 = np.float32
    bf = ml_dtypes.bfloat16
    f8 = ml_dtypes.float8_e4m3
    g1 = np.asarray(inputs["ln1_g"], f32)
    b1ln = np.asarray(inputs["ln1_b"], f32)
    g2 = np.asarray(inputs["ln2_g"], f32)
    b2ln = np.asarray(inputs["ln2_b"], f32)
    wq = np.asarray(inputs["wq"], f32)
    wk = np.asarray(inputs["wk"], f32)
    wv = np.asarray(inputs["wv"], f32)
    wo = np.asarray(inputs["wo"], f32)
    w1 = np.asarray(inputs["w1"], f32)
    w2 = np.asarray(inputs["w2"], f32)

    bias_rows = {
        "bq": (b1ln @ wq).astype(f32),
        "bk": (b1ln @ wk).astype(f32),
        "bv": (b1ln @ wv).astype(f32),
        "bo": np.asarray(inputs["bo"], f32),
        "b2": np.asarray(inputs["b2"], f32),
    }
    ln1b_nz = bool(np.any(b1ln))
    bff1 = (b2ln @ w1 + np.asarray(inputs["b1"], f32)).astype(f32)
    bff1_tile = np.ascontiguousarray(bff1.reshape(MC, P).T)  # [P, MC]

    wq_b = np.ascontiguousarray((g1[:, None] * wq).astype(bf))
    wk_b = np.ascontiguousarray((g1[:, None] * wk).astype(bf))
    wv_b = np.ascontiguousarray((g1[:, None] * wv).astype(bf))
    wo_b = np.ascontiguousarray(wo.astype(bf))
    # fp8 FFN weights, pre-scaled; packed for DoubleRow operands:
    #   w1p[p, m, c, i, q] = (g2*w1*W1S)[(2c+i)*128+p, m*128+q]  (lhsT)
    #   w2p[p, j, i, n]    = (w2*W2S)[(2j+i)*128+p, n]           (rhs)
    w1g = (g2[:, None] * w1 * W1S).astype(f8)
    w1_b = np.ascontiguousarray(
        w1g.reshape(DC // 2, 2, P, MC, P)
        .transpose(2, 3, 0, 1, 4)
        .reshape(P, MC * (DC // 2) * 2 * P)
    )
    w2s = (w2 * W2S).astype(f8)
    w2_b = np.ascontiguousarray(
        w2s.reshape(MC // 2, 2, P, D).transpose(2, 0, 1, 3).reshape(P, (MC // 2) * 2 * D)
    )

    Q = np.asarray(inputs["Q"], f32)
    K = np.asarray(inputs["K"], f32)
    V = np.asarray(inputs["V"], f32)

    in_maps = []
    for c in range(NCORES):
        b = c // 2
        r0 = (c % 2) * TQ
        if DEDUP:
            kslc = slice(r0, r0 + TKV)
        else:
            kslc = slice(0, N)
        m = {
            "q_tok": np.ascontiguousarray(Q[b, r0 : r0 + TQ]),
            "k_tok": np.ascontiguousarray(K[b, kslc].astype(bf)),
            "v_tok": np.ascontiguousarray(V[b, kslc].astype(bf)),
            "wq": wq_b,
            "wk": wk_b,
            "wv": wv_b,
            "wo": wo_b,
            "w1": w1_b,
            "w2": w2_b,
            "bff1": bff1_tile,
        }
        for name, row in bias_rows.items():
            if np.any(row):
                m["brow_" + name] = row[None, :].astype(bf)
        in_maps.append(m)
    return in_maps, bias_rows, ln1b_nz


_NC_CACHE = {}


def kernel(**inputs) -> np.ndarray:
    from concourse.bass_utils import run_bass_kernel_spmd

    in_maps, bias_rows, ln1b_nz = prep_inputs(inputs)
    bias_key = (ln1b_nz,) + tuple(
        sorted(n for n, r in bias_rows.items() if np.any(r))
    )
    if bias_key not in _NC_CACHE:
        _NC_CACHE[bias_key] = build_nc(bias_rows, ln1b_nz)
    nc = _NC_CACHE[bias_key]
    res = run_bass_kernel_spmd(nc, in_maps, core_ids=list(range(NCORES)))
    out = np.empty((B, N, D), np.float32)
    for c in range(NCORES):
        b = c // 2
        r0 = (c % 2) * TQ
        out[b, r0 : r0 + TQ] = res.results[c]["y"]
    return out
